# revision 37
# baseline (speedup 1.0000x reference)
"""Trainium2 Bass kernel for a 30-expert MLP ensemble.

Computes out[b] = mean_e sigmoid(relu(x @ W1[e] + b1[e]) @ W2[e] + b2[e])
for x [8192, 1024], W1 [30, 1024, 2048], W2 [30, 2048].

Strategy: data-parallel over the batch axis. Each of the 8 cores gets a
1024-row shard of x (pre-transposed on host) and the full replicated
weight stack. No collectives: the host concatenates the 8 disjoint
output shards.

All matmuls run in fp8 (e4m3) with MatmulPerfMode.DoubleRow: two
128-wide k-subtiles per matmul at 2 moving-rows/cycle, 2x the
fp32r/bf16 PE rate. W1 and W2 are scaled by 64 before the fp8 cast so
their U(-1/32..) values sit in e4m3's normal range; the 1/64 is folded
back in via the activation scale (out = act(psum/64 + bias)). PSUM
accumulates fp32, so the main precision loss is the ~2^-4 fp8
quantization of x, W1, relu(h), W2 — which averages down over the
1024/2048-long contractions and the 30-expert mean. The sigmoid
staging / expert-mean path runs in bf16 (its matmuls then go at 1
cycle/row instead of fp32's 4). Total ~3e-3 relative error
(tolerance 2e-2).

fc2 contracts hidden pairs (2jp, 2jp+1) per DoubleRow matmul. Its
stationary is w2 replicated across all 128 PE columns (the ldweights
ISA rejects narrow DoubleRow stationaries; 128 columns matches the
known-good fc1 shape), so all 128 PSUM partitions hold the same logit
row and sigmoid reads partition 0. The fc2 matmul for pair step N is
emitted after the fc1 group of step N+1, giving the Relu eviction a
full fc1 group (~2us) of slack so the PE never stalls on the Scalar
engine (stalls also drop the PE out of its max p-state).

Layouts (prepared host-side in numpy, fp8 = ml_dtypes.float8_e4m3):
  xt  [128, IB, BC]        xt[p,s,b]      = x[b, s*128 + p]
  w1  [E*JP, 128, 2, IB, 128] w1[gp,p,u,s,q] = 64*W1[e, s*128+p, (2jp+u)*128+q]
  b1  [128, E*JB]          b1[p, e*JB+jb] = b1[e, jb*128 + p]
  w2r [128, E*JP, 2, 128]  w2r[p,gp,u,q]  = 64*W2[e, (2jp+u)*128+p]
  b2  [1, E]

Scheduling: a few junk warmup matmuls raise the PE p-state during the
~10us DMA/boot lead-in; w1 is prefetched 4 pair-blocks deep; the replicated
w2 streams per expert two experts ahead; the expert mean runs as a
partial matmul over experts 0..28 during expert 29's fc1 (masked tail
matmul adds expert 29), so the post-loop tail is just
sigmoid -> 213ns matmul -> copy -> DMA.
"""

import numpy as np

import concourse.bacc as bacc
import concourse.mybir as mybir
import concourse.tile as tile
from concourse.bass_utils import run_bass_kernel_spmd

N_CORES = 8
P = 128
NB = 512  # matmul moving free dim (psum bank = 512 fp32)
SCALE = 64.0  # host-side premultiply of W1/W2 before the fp8 cast

E_FULL, I_FULL, H_FULL, B_FULL = 30, 1024, 2048, 8192


def build_bass(E=E_FULL, I=I_FULL, H=H_FULL, BC=B_FULL // N_CORES):
    IB = I // P
    JB = H // P
    JP = JB // 2
    BH = BC // NB
    f32 = mybir.dt.float32
    f8 = mybir.dt.float8e4
    DoubleRow = mybir.MatmulPerfMode.DoubleRow
    Relu = mybir.ActivationFunctionType.Relu
    Sigmoid = mybir.ActivationFunctionType.Sigmoid
    Copy = mybir.ActivationFunctionType.Copy

    nc = bacc.Bacc(None, target_bir_lowering=False)
    xt_d = nc.dram_tensor("xt", [P, IB, BC], f8, kind="ExternalInput")
    w1_d = nc.dram_tensor("w1", [E * JP, P, 2, IB, P], f8, kind="ExternalInput")
    b1_d = nc.dram_tensor("b1", [P, E * JB], f32, kind="ExternalInput")
    w2_d = nc.dram_tensor("w2", [P, E * JP, 2, P], f8, kind="ExternalInput")
    b2_d = nc.dram_tensor("b2", [1, E], f32, kind="ExternalInput")
    out_d = nc.dram_tensor("out", [1, BC], f32, kind="ExternalOutput")

    with tile.TileContext(nc) as tc:
        with (
            tc.tile_pool(name="const", bufs=1) as const_pool,
            tc.tile_pool(name="xt", bufs=1) as xt_pool,
            tc.tile_pool(name="w1", bufs=4) as w1_pool,
            tc.tile_pool(name="w2", bufs=4) as w2_pool,
            tc.tile_pool(name="h", bufs=6) as h_pool,
            tc.tile_pool(name="osb", bufs=2) as osb_pool,
            tc.tile_pool(name="fc1ps", bufs=4, space="PSUM") as fc1_psum,
            tc.tile_pool(name="fc2ps", bufs=4, space="PSUM") as fc2_psum,
        ):
            # DMA emission order below is chosen so the PE's first fc1
            # group can start ~2.5us in: b1, then the bh=0 half of x, then
            # the first two w1 blocks, and only then the rest. The bulky
            # replicated-w2 transfer (7.9 MB) is split per expert and
            # streamed just-in-time two experts ahead, so it never sits in
            # front of latency-critical loads on the DMA queue.
            b1_t = const_pool.tile([P, E * JB], f32)
            nc.sync.dma_start(b1_t[:], b1_d[:])

            # warm the PE out of its low p-state during the DMA lead-in:
            # junk f32 matmuls over b1 (first tensor to arrive), result unread
            wm_ps = fc1_psum.tile([1, E * JB], f32, tag="fc1", name="warm_ps")
            for wi in range(4):
                nc.tensor.matmul(
                    wm_ps[:],
                    b1_t[:, 0:1],
                    b1_t[:, 0:E * JB],
                    start=True,
                    stop=True,
                )

            xt_t = xt_pool.tile([P, IB, BC], f8)
            nc.sync.dma_start(xt_t[:, :, 0:NB], xt_d[:, :, 0:NB])

            w1_pre = {}

            def pre_w1(gpi):
                w_t = w1_pool.tile([P, 2, IB, P], f8, tag="w1", name=f"w1p_{gpi}")
                nc.sync.dma_start(w_t[:], w1_d[gpi])
                w1_pre[gpi] = w_t

            pre_w1(0)

            b2_t = const_pool.tile([1, E], f32)
            nc.sync.dma_start(b2_t[:], b2_d[:])
            bf16 = mybir.dt.bfloat16
            # expert-mean weights, split so experts 0..E-2 reduce early and
            # the last expert's sigmoid staging tile is added in the tail
            cA_t = const_pool.tile([E - 1, 1], bf16)
            nc.any.memset(cA_t[:], 1.0 / E)
            cb1_t = const_pool.tile([1, 1], bf16)
            nc.any.memset(cb1_t[:], 1.0 / E)
            o_all = const_pool.tile([E, BC], bf16)

            w2_by_e = {}

            def load_w2(e):
                w2e = w2_pool.tile([P, JP, 2, P], f8, tag="w2", name=f"w2_{e}")
                nc.sync.dma_start(w2e[:], w2_d[:, e * JP:(e + 1) * JP])
                w2_by_e[e] = w2e

            load_w2(0)
            for bh in range(1, BH):
                nc.sync.dma_start(
                    xt_t[:, :, bh * NB:(bh + 1) * NB],
                    xt_d[:, :, bh * NB:(bh + 1) * NB],
                )
            for gpi in range(1, 4):
                pre_w1(gpi)
            load_w2(1)

            fc2_ps_by_e = {}
            mean_ps = []

            def fc1_step(gp):
                """fc1 for hidden pair gp: 8 DoubleRow matmuls + 4 Relu
                evictions into the fp8 pair tiles h_ts[bh][:, u, :]."""
                e, jp = divmod(gp, JP)
                if e == E - 1 and jp == 2:
                    # partial mean over experts 0..E-2 while the last expert
                    # is still computing (two groups of slack past sigmoid
                    # E-2); the tail only adds expert E-1
                    for bh in range(BH):
                        mps = fc2_psum.tile(
                            [1, NB], f32, tag="fc2", name=f"meanps_{bh}"
                        )
                        nc.tensor.matmul(
                            mps[:],
                            cA_t[:],
                            o_all[0:E - 1, bh * NB:(bh + 1) * NB],
                            start=True,
                            stop=False,
                        )
                        mean_ps.append(mps)
                if jp == 0:
                    if e + 2 < E:
                        load_w2(e + 2)
                    fc2_ps_by_e[e] = [
                        fc2_psum.tile(
                            [P, NB], f32, tag="fc2", name=f"fc2ps_{e}_{bh}"
                        )
                        for bh in range(BH)
                    ]
                h_ts = [
                    h_pool.tile([P, 2, NB], f8, tag="h", name=f"h_{gp}_{bh}")
                    for bh in range(BH)
                ]
                if gp in w1_pre:
                    w_t = w1_pre.pop(gp)
                else:
                    w_t = w1_pool.tile(
                        [P, 2, IB, P], f8, tag="w1", name=f"w1_{gp}"
                    )
                    nc.sync.dma_start(w_t[:], w1_d[gp])
                for u in range(2):
                    jb = 2 * jp + u
                    col = e * JB + jb
                    pss = [
                        fc1_psum.tile(
                            [P, NB], f32, tag="fc1", name=f"fc1ps_{col}_{bh}"
                        )
                        for bh in range(BH)
                    ]
                    for bh in range(BH):
                        for sb in range(0, IB, 2):
                            nc.tensor.matmul(
                                pss[bh][:],
                                w_t[:, u, sb:sb + 2, :],
                                xt_t[:, sb:sb + 2, bh * NB:(bh + 1) * NB],
                                start=(sb == 0),
                                stop=(sb == IB - 2),
                                perf_mode=DoubleRow,
                            )
                        nc.scalar.activation(
                            h_ts[bh][:, u, :],
                            pss[bh][:],
                            Relu,
                            bias=b1_t[:, col:col + 1],
                            scale=1.0 / SCALE,
                        )
                return (e, jp, h_ts)

            def fc2_step(st):
                e, jp, h_ts = st
                for bh in range(BH):
                    nc.tensor.matmul(
                        fc2_ps_by_e[e][bh][:],
                        w2_by_e[e][:, jp, :, :],
                        h_ts[bh][:],
                        start=(jp == 0),
                        stop=(jp == JP - 1),
                        perf_mode=DoubleRow,
                    )

            o_last = []

            def sig_step(e):
                for bh in range(BH):
                    o_stage = osb_pool.tile(
                        [1, NB],
                        bf16,
                        tag="olast" if e == E - 1 else "ostage",
                        name=f"osig_{e}_{bh}",
                    )
                    nc.scalar.activation(
                        o_stage[:],
                        fc2_ps_by_e[e][bh][0:1, :],
                        Sigmoid,
                        bias=b2_t[0:1, e:e + 1],
                        scale=1.0 / SCALE,
                    )
                    if e == E - 1:
                        o_last.append(o_stage)
                    else:
                        nc.sync.dma_start(
                            o_all[e:e + 1, bh * NB:(bh + 1) * NB], o_stage[:]
                        )
                del fc2_ps_by_e[e]
                del w2_by_e[e]

            prev = None
            for gp in range(E * JP):
                st = fc1_step(gp)
                if prev is not None:
                    fc2_step(prev)
                    if prev[1] == JP - 1:
                        sig_step(prev[0])
                prev = st
            fc2_step(prev)
            sig_step(prev[0])

            for bh in range(BH):
                nc.tensor.matmul(
                    mean_ps[bh][:],
                    cb1_t[:],
                    o_last[bh][:],
                    start=False,
                    stop=True,
                )
                o_sb = osb_pool.tile([1, NB], f32)
                nc.scalar.activation(o_sb[:], mean_ps[bh][:], Copy)
                nc.sync.dma_start(out_d[0:1, bh * NB:(bh + 1) * NB], o_sb[:])
    nc.compile()
    return nc


def prep_inputs(x, W1, b1, W2, b2, E, I, H, BC):
    IB = I // P
    JB = H // P
    JP = JB // 2
    f8 = mybir.dt.np(mybir.dt.float8e4)
    w1_l = np.ascontiguousarray(
        (W1.astype(np.float32) * SCALE)
        .reshape(E, IB, P, JP, 2, P)
        .transpose(0, 3, 2, 4, 1, 5)
        .reshape(E * JP, P, 2, IB, P)
    ).astype(f8)
    b1_l = np.ascontiguousarray(
        b1.reshape(E, JB, P).transpose(2, 0, 1).reshape(P, E * JB), np.float32
    )
    # w2 replicated across all 128 stationary columns (see module docstring)
    w2_n = (
        (W2.astype(np.float32) * SCALE)
        .reshape(E, JP, 2, P)
        .transpose(3, 0, 1, 2)
        .reshape(P, E * JP, 2, 1)
        .astype(f8)
    )
    w2_l = np.ascontiguousarray(np.broadcast_to(w2_n, (P, E * JP, 2, P)))
    b2_l = np.ascontiguousarray(b2.reshape(1, E), np.float32)
    ca_l = np.full((E - 1, 1), 1.0 / E, np.float32)
    cb_l = np.zeros((E, 1), np.float32)
    cb_l[E - 1, 0] = 1.0 / E
    in_maps = []
    for c in range(N_CORES):
        xc = np.asarray(x[c * BC:(c + 1) * BC], np.float32)  # [BC, I]
        xt = np.ascontiguousarray(xc.reshape(BC, IB, P).transpose(2, 1, 0)).astype(f8)
        in_maps.append(
            {"xt": xt, "w1": w1_l, "b1": b1_l, "w2": w2_l, "b2": b2_l,
             "ca": ca_l, "cb": cb_l}
        )
    return in_maps


def run(x, W1, b1, W2, b2, trace=False):
    E, I, H = W1.shape
    BC = x.shape[0] // N_CORES
    in_maps = prep_inputs(x, W1, b1, W2, b2, E, I, H, BC)
    nc = build_bass(E=E, I=I, H=H, BC=BC)
    res = run_bass_kernel_spmd(nc, in_maps, list(range(N_CORES)), trace=trace)
    outs = [res.results[c]["out"].reshape(BC) for c in range(N_CORES)]
    full = np.concatenate(outs)[:, None].astype(np.float32)
    return full, res


def kernel(x, W1, b1, W2, b2):
    out, _ = run(
        np.asarray(x), np.asarray(W1), np.asarray(b1), np.asarray(W2), np.asarray(b2)
    )
    return out
